# revision 1
# baseline (speedup 1.0000x reference)
"""Trainium2 Bass kernel: BinConv(3x3, pad 1) + BatchNorm(train) + Hardtanh.

Data-parallel over the batch across 8 NeuronCores (4 images/core), weights and
BN params replicated; BN batch statistics all-reduced core-to-core with
remote_dma_broadcast (no ncfw collective on the critical path).

Key design points:
  - binarize x and W to +-0.5 in one DVE op (is_ge; subtract 0.5). BatchNorm is
    positively scale-invariant, so conv(+-0.5, +-0.5) = conv(+-1, +-1)/4
    normalizes identically (eps enters at var/16: ~1e-7 relative effect).
  - +-0.5 is exact in fp8e4; matmuls run fp8 with MatmulPerfMode.DoubleRow so
    one matmul contracts all 256 input channels (2 fp8 weights per PE cell).
    Measured 191ns (max p-state) to 230ns (mid) per 448-col matmul; the
    stream is ~99% dense on the tensor engine.
  - activations live in SBUF as flat zero-padded 58x58 frames [c=128, 2, 3376]
    (3376 = pad for DoubleRow's 16-byte half-stride rule). A PSUM tile of
    [128, 464] covers 8 output rows in padded-frame coords (8*58), which makes
    every conv tap's rhs window contiguous (offset (8ch+dy)*58+dx); the two
    garbage columns per row are never evicted.
  - conv outputs are quarter-integers <= 576 -> exactly representable in fp16;
    y is staged in SBUF fp16 between pass 1 and pass 2.
  - per-chunk sum/sumsq stats are fused into PSUM eviction via accum_out
    (DVE copy for sum, ACT Square for sumsq; evpool is 8-deep so the ACT
    squares never cross-wait on DVE eviction progress).
  - tap-0 W loads lead both DMA queues so the first matmul fires as soon as
    the ~10us engine preamble allows.
  - BN stats exchange: 7 single-slot remote_dma_broadcast sends (slot d ->
    same-device tpb^d, XOR-relative, disjoint DMA-lane pairs) sharing one
    remote semaphore (+2 each -> one wait >= 14). Descriptors only encode
    addresses, so they are pre-generated in an early gpsimd tile_critical
    (no_gpsimd_drain=True: a drain would reset the pending ring) against a
    staging tile; when local stats land, a second critical copies
    loc->staging, fires one trigger_dma(count=7), waits, reduces, and does
    mean/var — all on gpsimd. tile_critical is required: Tile's scheduling
    sim cannot satisfy cross-core semaphore waits. No entry barrier: remote
    writes land >100us after launch while peers clear sems in the first
    ~10us. The UNWAITED 1-byte prelude AllGather registered before compile
    keeps NRT launching the 8 cores synchronized (without any collective in
    the NEFF they launch ms-staggered) while ncfw's 65-150us cold start
    stays off the critical path.
  - the ACT table for Abs_reciprocal_sqrt is pre-warmed right after pass 1
    so no table load lands on the critical path; scl/bia on DVE.
  - pass 2 is output-DMA-bound (6.4MB fp16 at the ~358GB/s per-core cap,
    ~18us): DVE takes 4 affines + all 8 clips, ACT the other 4 affines
    (gpsimd idle — it amplifies SBUF contention); whole-frame DMAs on the
    idle sync queue; the first unit is halved so the DMA pipe starts ~1us in.
  - host passes x/W pre-cast to bf16 (sign-exact vs f32) and W pre-laid-out as
    [c_lo=128, c_hi=2, tap=9, k]; output leaves the device as fp16 (values are
    clipped to [-1,1]; ~5e-4 quantization) and is cast to f32 on host.
"""


from contextlib import ExitStack

import numpy as np

import concourse.bacc as bacc
import concourse.tile as tile
from concourse import mybir

F32 = mybir.dt.float32
BF16 = mybir.dt.bfloat16
F16 = mybir.dt.float16
FP8 = mybir.dt.float8e4
AF = mybir.ActivationFunctionType
ALU = mybir.AluOpType

EPS = 1e-5
C = 256
K = 256
H = 56
HP = 58
SP = HP * HP  # 3364
SPPAD = 3376  # % 16 == 0 for DoubleRow half-stride
NCHUNK = 7  # chunks of 8 rows
ROWS = 8
WIN = ROWS * HP  # 464 contiguous window per chunk
HH = H // 2  # row-half for split loads


def build(n_cores: int, nimg: int, total_imgs: int, comm: str = "rdma"):
    """Build the per-core SPMD kernel. nimg = images per core.

    comm: "rdma" = XOR-butterfly all-reduce of the BN stats over
    remote_dma_broadcast (SBUF->peer-SBUF, ~us-scale); "allgather" = ncfw
    collective fallback (~25us+ per op on this runtime).
    """
    nc = bacc.Bacc("TRN2", target_bir_lowering=False, debug=False, num_devices=n_cores)

    x_h = nc.dram_tensor("x", [nimg, C, H, H], BF16, kind="ExternalInput")
    # W host-transposed/interleaved to [c_lo=128, c_hi=2, tap=9, k=K]
    w_h = nc.dram_tensor("w", [128, 2, 9, K], BF16, kind="ExternalInput")
    gamma_h = nc.dram_tensor("gamma", [K, 1], F32, kind="ExternalInput")
    beta_h = nc.dram_tensor("beta", [K, 1], F32, kind="ExternalInput")
    out_h = nc.dram_tensor("out", [nimg, K, H, H], F16, kind="ExternalOutput")

    inv_cnt = 1.0 / float(total_imgs * H * H)

    with ExitStack() as ctx:
        tc = ctx.enter_context(tile.TileContext(nc))
        singles = ctx.enter_context(tc.tile_pool(name="singles", bufs=1))
        wtmp = ctx.enter_context(tc.tile_pool(name="wtmp", bufs=1))
        xpool = ctx.enter_context(tc.tile_pool(name="xpool", bufs=3))
        xppool = ctx.enter_context(tc.tile_pool(name="xppool", bufs=3))
        ypool = ctx.enter_context(tc.tile_pool(name="ypool", bufs=1))
        # 8-deep: with only 3 bufs the ACT squares' scratch rotation makes
        # ACT cross-wait on DVE eviction progress, serializing the
        # end-of-pass-1 stats chain
        evpool = ctx.enter_context(tc.tile_pool(name="evpool", bufs=8))
        obpool = ctx.enter_context(tc.tile_pool(name="obpool", bufs=4))
        psum = ctx.enter_context(tc.tile_pool(name="psum", bufs=8, space="PSUM"))
        warmd = ctx.enter_context(tc.tile_pool(name="warmd", bufs=6, space="DRAM"))
        ccd = ctx.enter_context(tc.tile_pool(name="ccd", bufs=2, space="DRAM"))

        HQ = H // 4

        # ---- startup: tap-0 weights first on BOTH queues, then x q0 ----
        wraw = wtmp.tile([128, 2, 9, K], BF16)
        wfp8 = singles.tile([128, 2, 9, K], FP8)
        nc.sync.dma_start(out=wraw[:, 0, 0:1], in_=w_h[:, 0, 0:1])
        nc.scalar.dma_start(out=wraw[:, 1, 0:1], in_=w_h[:, 1, 0:1])

        # x image 0 q0 next on the scalar queue
        xr0 = []
        for ct in range(2):
            xr = xpool.tile([128, H, H], BF16, tag="xr", name=f"xr0_{ct}")
            nc.scalar.dma_start(
                out=xr[:, 0:HQ, :], in_=x_h[0, ct * 128 : (ct + 1) * 128, 0:HQ, :]
            )
            xr0.append(xr)
        # remaining W taps on sync, remaining x-0 quarters on scalar
        for t0, t1 in ((1, 5), (5, 9)):
            for ci in range(2):
                nc.sync.dma_start(out=wraw[:, ci, t0:t1], in_=w_h[:, ci, t0:t1])
        for q in (1, 2, 3):
            for ct in range(2):
                nc.scalar.dma_start(
                    out=xr0[ct][:, q * HQ : (q + 1) * HQ, :],
                    in_=x_h[0, ct * 128 : (ct + 1) * 128, q * HQ : (q + 1) * HQ, :],
                )

        def load_x(n, eng=None):
            eng = eng or nc.sync
            xrs = []
            for ct in range(2):
                xr = xpool.tile([128, H, H], BF16, tag="xr", name=f"xr{n}_{ct}")
                for q in range(4):
                    eng.dma_start(
                        out=xr[:, q * HQ : (q + 1) * HQ, :],
                        in_=x_h[
                            n, ct * 128 : (ct + 1) * 128, q * HQ : (q + 1) * HQ, :
                        ],
                    )
                xrs.append(xr)
            return xrs

        def w_binarize(t0, t1):
            for ci in range(2):
                nc.vector.tensor_scalar(
                    out=wfp8[:, ci, t0:t1],
                    in0=wraw[:, ci, t0:t1],
                    scalar1=0.0,
                    scalar2=0.5,
                    op0=ALU.is_ge,
                    op1=ALU.subtract,
                )

        # image-0 frame: borders zeroed first (pure DVE, no DMA deps)
        xp0 = xppool.tile([128, 2, SPPAD], FP8, tag="xp", name="xp0")
        v0 = [xp0[:, ct, :SP].rearrange("p (h w) -> p h w", w=HP) for ct in range(2)]
        for ct in range(2):
            v = v0[ct]
            nc.vector.memset(v[:, 0, :], 0.0)
            nc.vector.memset(v[:, HP - 1, :], 0.0)
            nc.vector.memset(v[:, 1 : HP - 1, 0:1], 0.0)
            nc.vector.memset(v[:, 1 : HP - 1, HP - 1 : HP], 0.0)

        def bin_quarter(q):
            for ct in range(2):
                nc.vector.tensor_scalar(
                    out=v0[ct][:, 1 + q * HQ : 1 + (q + 1) * HQ, 1 : HP - 1],
                    in0=xr0[ct][:, q * HQ : (q + 1) * HQ, :],
                    scalar1=0.0,
                    scalar2=0.5,
                    op0=ALU.is_ge,
                    op1=ALU.subtract,
                )

        # DVE startup order: tap-0 weights, then first x quarter, then the rest
        w_binarize(0, 1)
        bin_quarter(0)
        w_binarize(1, 5)
        bin_quarter(1)
        w_binarize(5, 9)
        bin_quarter(2)
        bin_quarter(3)

        eps_t = singles.tile([128, 1], F32)
        nc.vector.memset(eps_t[:], EPS)

        gam = singles.tile([128, 2], F32)
        bet = singles.tile([128, 2], F32)
        for kt in range(2):
            nc.gpsimd.dma_start(
                out=gam[:, kt : kt + 1], in_=gamma_h[kt * 128 : (kt + 1) * 128, :]
            )
            nc.gpsimd.dma_start(
                out=bet[:, kt : kt + 1], in_=beta_h[kt * 128 : (kt + 1) * 128, :]
            )

        # ---------------- pass 1: conv + stats ----------------
        ysb = [
            ypool.tile([128, nimg, NCHUNK, ROWS, H], F16, name=f"ysb{kt}")
            for kt in range(2)
        ]
        sumc = singles.tile([128, 2, nimg * NCHUNK], F32)
        sqc = singles.tile([128, 2, nimg * NCHUNK], F32)

        def binarize(n, xrs):
            xp = xppool.tile([128, 2, SPPAD], FP8, tag="xp", name=f"xp{n}")
            for ct in range(2):
                v = xp[:, ct, :SP].rearrange("p (h w) -> p h w", w=HP)
                # zero border + tail pad; interior fully overwritten
                nc.vector.memset(v[:, 0, :], 0.0)
                nc.vector.memset(v[:, HP - 1, :], 0.0)
                nc.vector.memset(v[:, 1 : HP - 1, 0:1], 0.0)
                nc.vector.memset(v[:, 1 : HP - 1, HP - 1 : HP], 0.0)
                for q in range(4):
                    nc.vector.tensor_scalar(
                        out=v[:, 1 + q * HQ : 1 + (q + 1) * HQ, 1 : HP - 1],
                        in0=xrs[ct][:, q * HQ : (q + 1) * HQ, :],
                        scalar1=0.0,
                        scalar2=0.5,
                        op0=ALU.is_ge,
                        op1=ALU.subtract,
                    )
            return xp

        xp_cur = xp0
        for n in range(nimg):
            xp_use, xp_cur = xp_cur, None
            if n + 1 < nimg:
                xp_cur = binarize(n + 1, load_x(n + 1))

            xpv = xp_use[:, :, :SP].rearrange("p i (h w) -> p i h w", w=HP)
            for kt in range(2):
                banks = [
                    psum.tile([128, ROWS, H], F32, tag="ps", name=f"ps{n}_{kt}_{ch}")
                    for ch in range(NCHUNK)
                ]
                # tap-major: measured evictions complete sooner after the
                # closing matmuls than chunk-major (coalesced tensor->DVE
                # semaphore updates penalize per-chunk completion waits)
                for t9 in range(9):
                    dy, dx = divmod(t9, 3)
                    for ch in range(NCHUNK):
                        r0 = ROWS * ch + dy
                        nc.tensor.matmul(
                            banks[ch][:],
                            wfp8[:, :, t9, kt * 128 : (kt + 1) * 128],
                            xpv[:, :, r0 : r0 + ROWS, dx : dx + H],
                            start=(t9 == 0),
                            stop=(t9 == 8),
                            perf_mode=mybir.MatmulPerfMode.DoubleRow,
                        )
                for ch in range(NCHUNK):
                    col = n * NCHUNK + ch
                    psv = banks[ch][:]
                    # evict valid columns to fp16 (exact) + per-chunk sum on DVE
                    nc.vector.tensor_scalar(
                        out=ysb[kt][:, n, ch],
                        in0=psv,
                        scalar1=1.0,
                        scalar2=0.0,
                        op0=ALU.mult,
                        op1=ALU.add,
                        accum_out=sumc[:, kt, col : col + 1],
                    )
                    # sum of squares on ACT
                    sqs = evpool.tile([128, ROWS, H], F32, tag="sqs")
                    nc.scalar.activation(
                        out=sqs[:],
                        in_=psv,
                        func=AF.Square,
                        accum_out=sqc[:, kt, col : col + 1],
                    )

        # pre-warm the ACT rsqrt table while the stats exchange is in flight
        tblw = singles.tile([128, 1], F32)
        nc.scalar.activation(
            out=tblw[:], in_=eps_t[:], func=AF.Abs_reciprocal_sqrt, bias=eps_t[:]
        )

        # ---------------- stats reduce + exchange ----------------
        loc = singles.tile([128, 4], F32)
        fold_scr = singles.tile([128, nimg * NCHUNK], F32)

        def issue_folds(eng):
            for kt in range(2):
                eng.tensor_scalar(
                    out=fold_scr[:],
                    in0=sumc[:, kt, :],
                    scalar1=1.0,
                    scalar2=0.0,
                    op0=ALU.mult,
                    op1=ALU.add,
                    accum_out=loc[:, kt : kt + 1],
                )
                eng.tensor_scalar(
                    out=fold_scr[:],
                    in0=sqc[:, kt, :],
                    scalar1=1.0,
                    scalar2=0.0,
                    op0=ALU.mult,
                    op1=ALU.add,
                    accum_out=loc[:, 2 + kt : 3 + kt],
                )

        if comm == "rdma":
            # Single-phase all-to-all over remote_dma_broadcast: 7 single-slot
            # sends (slot d -> tpb^d, disjoint DMA-lane pairs, shared remote
            # sem: +2 per arrival -> wait >= 14 once), so cross-core skew is
            # paid once instead of per-butterfly-round. Descriptors only
            # encode ADDRESSES, so they are pre-generated in an early critical
            # section against a staging tile (sloc) while pass 1 runs; when
            # the local stats land, the second critical section copies
            # loc->sloc, fires one trigger for all 7 preps, waits, reduces.
            # No entry barrier wait: remote writes land >120us after launch
            # while peers clear sems in the first ~10us; the unwaited prelude
            # AllGather registered below keeps NRT launches synchronized.
            # All on gpsimd inside tile_critical so Tile's scheduling sim
            # doesn't try (and fail) to satisfy the cross-core sem waits.
            sloc = singles.tile([128, 4], F32, name="a2a_src")
            rall = singles.tile([128, 7, 4], F32, name="a2a_rbuf")
            gstat = singles.tile([128, 4], F32, name="a2a_gstat")
            mv = singles.tile([128, 4], F32)
            rsem = nc.alloc_semaphore(name="a2a_r")
            lsem = nc.alloc_semaphore(name="a2a_l")
            psem = nc.alloc_semaphore(name="a2a_p")
            with tc.tile_critical(
                sync_engine=mybir.EngineType.Pool, no_gpsimd_drain=True
            ):
                for d in range(1, 8):
                    rdests = [None] * 8
                    rdests[d] = (0, d)
                    nc.gpsimd.remote_dma_broadcast(
                        out_ap=rall[:, d - 1, :],
                        in_ap=sloc[:],
                        remote_sem=rsem,
                        local_sem=lsem,
                        rdests=rdests,
                    ).then_inc(psem, 1)
                nc.gpsimd.wait_ge(psem, 7)
            # folds on DVE (accum_out is not in the Pool engine's ISA)
            issue_folds(nc.vector)
            with tc.tile_critical(sync_engine=mybir.EngineType.Pool):
                nc.gpsimd.tensor_scalar(
                    out=sloc[:],
                    in0=loc[:],
                    scalar1=1.0,
                    scalar2=0.0,
                    op0=ALU.mult,
                    op1=ALU.add,
                )
                nc.gpsimd.trigger_dma(count=7)
                nc.gpsimd.wait_ge(rsem, 14)
                nc.gpsimd.tensor_add(
                    out=rall[:, 0:3, :], in0=rall[:, 0:3, :], in1=rall[:, 3:6, :]
                )
                nc.gpsimd.tensor_add(
                    out=rall[:, 0, :], in0=rall[:, 0, :], in1=rall[:, 1, :]
                )
                nc.gpsimd.tensor_add(
                    out=rall[:, 0, :], in0=rall[:, 0, :], in1=rall[:, 2, :]
                )
                nc.gpsimd.tensor_add(
                    out=rall[:, 0, :], in0=rall[:, 0, :], in1=rall[:, 6, :]
                )
                nc.gpsimd.tensor_add(
                    out=gstat[:], in0=rall[:, 0, :], in1=loc[:]
                )
                # mean/var scaling on gpsimd too: saves a cross-engine hop
                # before the ACT rsqrt
                nc.gpsimd.tensor_scalar(
                    out=mv[:],
                    in0=gstat[:],
                    scalar1=inv_cnt,
                    scalar2=None,
                    op0=ALU.mult,
                )
                nc.gpsimd.tensor_mul(
                    out=gstat[:, 0:2], in0=mv[:, 0:2], in1=mv[:, 0:2]
                )
                nc.gpsimd.tensor_sub(
                    out=mv[:, 2:4], in0=mv[:, 2:4], in1=gstat[:, 0:2]
                )
        else:
            issue_folds(nc.vector)
            gstat = singles.tile([128, 4], F32)
            cc_in = ccd.tile([128, 4], F32)
            cc_out = ccd.tile([n_cores * 128, 4], F32, addr_space="Shared")
            nc.gpsimd.dma_start(out=cc_in[:], in_=loc[:])
            nc.gpsimd.collective_compute(
                "AllGather",
                ALU.bypass,
                replica_groups=[list(range(n_cores))],
                ins=[cc_in.opt()],
                outs=[cc_out.opt()],
            )
            # gather all ranks' stats to SBUF then reduce locally
            allst = singles.tile([128, n_cores, 4], F32)
            nc.gpsimd.dma_start(
                out=allst[:],
                in_=cc_out.rearrange("(r p) c -> p r c", p=128),
            )
            h = n_cores // 2
            while h > 1:
                nc.vector.tensor_add(
                    out=allst[:, 0:h, :], in0=allst[:, 0:h, :], in1=allst[:, h : 2 * h, :]
                )
                h //= 2
            nc.vector.tensor_add(
                out=gstat[:], in0=allst[:, 0, :], in1=allst[:, 1, :]
            )

        # ---------------- scale/bias (DVE except the rsqrt) ----------------
        if comm != "rdma":
            mv = singles.tile([128, 4], F32)
            nc.vector.tensor_scalar(
                out=mv[:], in0=gstat[:], scalar1=inv_cnt, scalar2=None, op0=ALU.mult
            )
            m2 = singles.tile([128, 2], F32)
            nc.vector.tensor_mul(out=m2[:], in0=mv[:, 0:2], in1=mv[:, 0:2])
            nc.vector.tensor_sub(out=mv[:, 2:4], in0=mv[:, 2:4], in1=m2[:])
        mean = mv[:, 0:2]
        var = mv[:, 2:4]
        rstd = singles.tile([128, 2], F32)
        nc.scalar.activation(
            out=rstd[:], in_=var, func=AF.Abs_reciprocal_sqrt, bias=eps_t[:]
        )
        scl = singles.tile([128, 2], F32)
        nc.vector.tensor_mul(out=scl[:], in0=gam[:], in1=rstd[:])
        bia = singles.tile([128, 2], F32)
        nc.vector.tensor_mul(out=bia[:], in0=mean[:], in1=scl[:])
        nc.vector.tensor_sub(out=bia[:], in0=bet[:], in1=bia[:])

        # ---------------- pass 2: affine + clip + DMA out ----------------
        # 8 units of [128, 3136] fp16; 2 elementwise passes each (affine, clip).
        # DVE (0.36 ns/elem) takes 4 affines + all 8 clips, ACT (0.91) the
        # other 4 affines; gpsimd stays idle (slow + amplifies SBUF
        # contention). DMA (6.4MB out, ~18us) is the pacer, so unit 0 is
        # halved to start it early; whole-frame DMAs all go on the sync queue
        # (idle in pass 2; triggers are ~0.6us vs ~2.2us transfers).
        NFULL = NCHUNK * ROWS * H
        NHALF = NFULL // 2
        aff_dve = {0, 2, 4, 6}
        clip_gps = set()
        unit = 0
        for n in range(nimg):
            for kt in range(2):
                ob = obpool.tile([128, NFULL], F16, tag="ob")
                ysrc = ysb[kt][:, n].rearrange("p a b c -> p (a b c)")
                obv = ob[:].rearrange("p (a b) -> p a b", b=H)
                dma_eng = nc.sync
                if unit == 0:
                    # halves so the first output DMA fires ~1us into pass 2
                    for hf in range(2):
                        sl = slice(hf * NHALF, (hf + 1) * NHALF)
                        nc.vector.tensor_scalar(
                            out=ob[:, sl],
                            in0=ysrc[:, sl],
                            scalar1=scl[:, kt : kt + 1],
                            scalar2=bia[:, kt : kt + 1],
                            op0=ALU.mult,
                            op1=ALU.add,
                        )
                        nc.vector.tensor_scalar(
                            out=ob[:, sl],
                            in0=ob[:, sl],
                            scalar1=1.0,
                            scalar2=-1.0,
                            op0=ALU.min,
                            op1=ALU.max,
                        )
                        dma_eng.dma_start(
                            out=out_h[
                                n,
                                kt * 128 : (kt + 1) * 128,
                                hf * HH : (hf + 1) * HH,
                                :,
                            ],
                            in_=obv[:, hf * HH : (hf + 1) * HH, :],
                        )
                    unit += 1
                    continue
                if unit in aff_dve:
                    nc.vector.tensor_scalar(
                        out=ob[:],
                        in0=ysrc,
                        scalar1=scl[:, kt : kt + 1],
                        scalar2=bia[:, kt : kt + 1],
                        op0=ALU.mult,
                        op1=ALU.add,
                    )
                else:
                    nc.scalar.activation(
                        out=ob[:],
                        in_=ysrc,
                        func=AF.Identity,
                        bias=bia[:, kt : kt + 1],
                        scale=scl[:, kt : kt + 1],
                    )
                clip_eng = nc.gpsimd if unit in clip_gps else nc.vector
                clip_eng.tensor_scalar(
                    out=ob[:],
                    in0=ob[:],
                    scalar1=1.0,
                    scalar2=-1.0,
                    op0=ALU.min,
                    op1=ALU.max,
                )
                dma_eng.dma_start(
                    out=out_h[n, kt * 128 : (kt + 1) * 128, :, :],
                    in_=obv[:],
                )
                unit += 1

    if comm == "rdma":
        # Register the kernel-entry barrier replica groups WITHOUT emitting a
        # wait: compile() then inserts a 1-byte prelude AllGather and sets
        # has_collectives, which makes NRT bring up global comm and launch
        # the 8 cores synchronized (without any collective in the NEFF the
        # cores launch ms-staggered and the butterfly eats the whole skew).
        # Nobody waits on it, so ncfw's 65-150us cold start stays off the
        # critical path entirely.
        nc._bir_kernel_barrier_sem_replica_groups.extend([set(range(n_cores))])

    nc.compile()
    return nc


def prep_w(W):
    """Host layout prep: W [K,C,3,3] -> [c_lo=128, c_hi=2, tap=9, K] f32."""
    wt = W.astype(np.float32).transpose(1, 2, 3, 0).reshape(C, 9, K)  # [c, t, k]
    return np.ascontiguousarray(wt.reshape(2, 128, 9, K).transpose(1, 0, 2, 3))


def _ensure_ntff_hooks():
    """Make run_bass_kernel_spmd's trace path importable on images whose
    antenv lacks axon_hooks (bass_utils hard-imports it when BASS_TRACE is
    set). Registers the real ctypes hook when available, else a None hook
    (bass_utils then logs and skips tracing instead of crashing)."""
    import sys
    import types

    try:
        import antenv
    except ImportError:
        return
    if hasattr(antenv, "axon_hooks") or "antenv.axon_hooks" in sys.modules:
        return
    hook = None
    try:
        from trn_agent_boot.trn_boot import _ntff_profile_via_ctypes

        hook = _ntff_profile_via_ctypes("/opt/axon/libaxon_pjrt.so")
    except Exception:
        hook = None
    mod = types.ModuleType("antenv.axon_hooks")
    mod.get_axon_ntff_profile_hook = lambda: hook
    mod.set_axon_ntff_profile_hook = lambda h: None
    sys.modules["antenv.axon_hooks"] = mod
    antenv.axon_hooks = mod


_ensure_ntff_hooks()


_CACHE = {}


def _get_compiled():
    if "nc" not in _CACHE:
        _CACHE["nc"] = build(8, 4, 32)
    return _CACHE["nc"]


def kernel(x, W, gamma, beta):
    """Full-input entry point: shard batch over 8 cores, run SPMD, gather."""
    import ml_dtypes
    from concourse.bass_utils import run_bass_kernel_spmd

    n_cores, nimg = 8, 4
    nc = _get_compiled()
    w2 = prep_w(np.asarray(W)).astype(ml_dtypes.bfloat16)
    g2 = np.ascontiguousarray(np.asarray(gamma, np.float32).reshape(K, 1))
    b2 = np.ascontiguousarray(np.asarray(beta, np.float32).reshape(K, 1))
    xb = np.asarray(x).astype(ml_dtypes.bfloat16)
    in_maps = [
        {
            "x": np.ascontiguousarray(xb[c * nimg : (c + 1) * nimg]),
            "w": w2,
            "gamma": g2,
            "beta": b2,
        }
        for c in range(n_cores)
    ]
    res = run_bass_kernel_spmd(nc, in_maps, core_ids=list(range(n_cores)))
    out = np.concatenate(
        [res.results[c]["out"] for c in range(n_cores)], axis=0
    ).astype(np.float32)
    return out


def run_traced(x, W, gamma, beta):
    """Like kernel() but with NTFF tracing; returns (out, BassKernelResults)."""
    import ml_dtypes
    from concourse.bass_utils import run_bass_kernel_spmd

    n_cores, nimg = 8, 4
    nc = _get_compiled()
    w2 = prep_w(np.asarray(W)).astype(ml_dtypes.bfloat16)
    g2 = np.ascontiguousarray(np.asarray(gamma, np.float32).reshape(K, 1))
    b2 = np.ascontiguousarray(np.asarray(beta, np.float32).reshape(K, 1))
    xb = np.asarray(x).astype(ml_dtypes.bfloat16)
    in_maps = [
        {
            "x": np.ascontiguousarray(xb[c * nimg : (c + 1) * nimg]),
            "w": w2,
            "gamma": g2,
            "beta": b2,
        }
        for c in range(n_cores)
    ]
    res = run_bass_kernel_spmd(nc, in_maps, core_ids=list(range(n_cores)), trace=True)
    out = np.concatenate(
        [res.results[c]["out"] for c in range(n_cores)], axis=0
    ).astype(np.float32)
    return out, res



# revision 2
# speedup vs baseline: 1.0981x; 1.0981x over previous
"""Trainium2 Bass kernel: BinConv(3x3, pad 1) + BatchNorm(train) + Hardtanh.

Data-parallel over the batch across 8 NeuronCores (4 images/core), weights and
BN params replicated; BN batch statistics all-reduced core-to-core with
remote_dma_broadcast (no ncfw collective on the critical path).

v2 restructure (vs the 211-225us baseline): the 8 core launches are staggered
~50us by the PJRT dispatch, so a single end-of-conv stats exchange stalls the
early cores ~55us with every engine idle. Fixes:
  - x and W arrive pre-binarized AND pre-padded from the host as fp8 frames
    (+-1 exact in fp8e4; W +-0.5). No device-side binarize at all: the input
    pipeline is pure DMA (3.4MB vs 6.4MB bf16), the first matmul fires ~2us
    in, and the DVE is free during pass 1.
  - the conv runs kt-outer (output-channel half), img-inner. Each kt half's
    BN stats fold+send fires the moment that half's conv is done (~t+60us for
    kt0), so the kt0 exchange crosses the wire while kt1's conv still runs.
    Pass 2 for kt0 (affine+clip+DMA-out) fills the former idle window. Only
    kt1's tail (~12us: 1KB stats xfer + affine/clip pipeline + 3.2MB out-DMA)
    pays the launch skew.
  - gpsimd program order: pregen(14 descs), send0, send1, recv0, recv1 —
    send1 sits ahead of recv0 so a late kt0 peer can never delay this core's
    kt1 send (the skew is paid exactly once, at recv1).

Carried over from the baseline design:
  - conv(+-1, +-0.5) = conv(+-1,+-1)/2; BatchNorm is positively
    scale-invariant, so normalization is identical (eps enters at var/4).
  - fp8 matmuls with MatmulPerfMode.DoubleRow contract all 256 input channels
    in one pass; activations live in SBUF as flat zero-padded 58x58 frames
    [c=128, 2, 3376] (3376 = pad for DoubleRow's 16-byte half-stride rule).
    A PSUM tile of [128, 8, 56] covers 8 output rows; every tap's rhs window
    is one strided AP (offset (8ch+dy)*58+dx).
  - conv outputs are half-integers <= 1152 -> exact in fp16; y stages in SBUF
    fp16 between passes. Per-chunk sum/sumsq stats fuse into PSUM eviction
    via accum_out (DVE copy for sum, ACT Square for sumsq; evpool 8-deep).
  - stats exchange per kt: 7 single-slot remote_dma_broadcast sends (slot d ->
    same-device tpb^d, XOR-relative), shared remote sem (+2 each -> one wait
    >= 14). Descriptors only encode addresses, so all 14 (7 per kt) are
    pre-generated in an early gpsimd tile_critical (no_gpsimd_drain=True —
    a drain would reset the pending ring) against staging tiles; each kt's
    send critical copies loc->staging and fires trigger_dma(count=7).
    tile_critical is required: Tile's scheduling sim cannot satisfy
    cross-core semaphore waits. The UNWAITED 1-byte prelude AllGather
    registered before compile keeps NRT launching the 8 cores synchronized
    (without any collective in the NEFF they launch ms-staggered) while
    ncfw's 65-150us cold start stays off the critical path.
  - the ACT table for Abs_reciprocal_sqrt is pre-warmed right after the conv
    so no table load lands on the critical path.
  - output leaves the device as fp16 (values clipped to [-1,1]; ~5e-4
    quantization) and is cast to f32 on host.
"""


from contextlib import ExitStack

import numpy as np

import concourse.bacc as bacc
import concourse.tile as tile
from concourse import mybir

F32 = mybir.dt.float32
F16 = mybir.dt.float16
FP8 = mybir.dt.float8e4
AF = mybir.ActivationFunctionType
ALU = mybir.AluOpType

EPS = 1e-5
C = 256
K = 256
H = 56
HP = 58
SP = HP * HP  # 3364
SPPAD = 3376  # % 16 == 0 for DoubleRow half-stride
NCHUNK = 7  # chunks of 8 rows
ROWS = 8
HH = H // 2


def build(n_cores: int, nimg: int, total_imgs: int):
    """Build the per-core SPMD kernel. nimg = images per core."""
    nc = bacc.Bacc("TRN2", target_bir_lowering=False, debug=False, num_devices=n_cores)

    # x pre-binarized (+-1) and pre-padded to 58x58 frames on host
    x_h = nc.dram_tensor("x", [nimg, 128, 2, SPPAD], FP8, kind="ExternalInput")
    # W pre-binarized (+-0.5), host-interleaved to [c_lo=128, tap=9, c_hi=2, k]
    w_h = nc.dram_tensor("w", [128, 9, 2, K], FP8, kind="ExternalInput")
    gamma_h = nc.dram_tensor("gamma", [K, 1], F32, kind="ExternalInput")
    beta_h = nc.dram_tensor("beta", [K, 1], F32, kind="ExternalInput")
    out_h = nc.dram_tensor("out", [nimg, K, H, H], F16, kind="ExternalOutput")

    inv_cnt = 1.0 / float(total_imgs * H * H)

    with ExitStack() as ctx:
        tc = ctx.enter_context(tile.TileContext(nc))
        singles = ctx.enter_context(tc.tile_pool(name="singles", bufs=1))
        # 8-deep: with fewer bufs the ACT squares' scratch rotation makes ACT
        # cross-wait on DVE eviction progress, serializing the stats chain
        evpool = ctx.enter_context(tc.tile_pool(name="evpool", bufs=8))
        obpool = ctx.enter_context(tc.tile_pool(name="obpool", bufs=4))
        obpool2 = ctx.enter_context(tc.tile_pool(name="obpool2", bufs=4))
        psum = ctx.enter_context(tc.tile_pool(name="psum", bufs=8, space="PSUM"))

        # ---- startup: tap-0 weights first on sync, then frames on both ----
        wfp8 = singles.tile([128, 9, 2, K], FP8)
        nc.sync.dma_start(out=wfp8[:, 0:1], in_=w_h[:, 0:1])

        xpf = singles.tile([128, nimg, 2, SPPAD], FP8)
        # image 0 first (both cts in parallel on the two queues), then W tail,
        # then the remaining images
        nc.scalar.dma_start(out=xpf[:, 0, 0], in_=x_h[0, :, 0])
        nc.sync.dma_start(out=xpf[:, 0, 1], in_=x_h[0, :, 1])
        nc.sync.dma_start(out=wfp8[:, 1:9], in_=w_h[:, 1:9])
        for n in range(1, nimg):
            nc.scalar.dma_start(out=xpf[:, n, 0], in_=x_h[n, :, 0])
            nc.sync.dma_start(out=xpf[:, n, 1], in_=x_h[n, :, 1])

        eps_t = singles.tile([128, 1], F32)
        nc.vector.memset(eps_t[:], EPS)

        gam = singles.tile([128, 2], F32)
        bet = singles.tile([128, 2], F32)
        for kt in range(2):
            nc.gpsimd.dma_start(
                out=gam[:, kt : kt + 1], in_=gamma_h[kt * 128 : (kt + 1) * 128, :]
            )
            nc.gpsimd.dma_start(
                out=bet[:, kt : kt + 1], in_=beta_h[kt * 128 : (kt + 1) * 128, :]
            )

        # ---------------- pass 1: conv + stats, kt-outer ----------------
        ysb = [
            singles.tile([128, nimg, NCHUNK, ROWS, H], F16, name=f"ysb{kt}")
            for kt in range(2)
        ]
        sumc = singles.tile([128, 2, nimg * NCHUNK], F32)
        sqc = singles.tile([128, 2, nimg * NCHUNK], F32)
        loc = [singles.tile([128, 2], F32, name=f"loc{kt}") for kt in range(2)]
        fold_scr = singles.tile([128, nimg * NCHUNK], F32)

        for kt in range(2):
            for n in range(nimg):
                xpv = xpf[:, n, :, :SP].rearrange("p i (h w) -> p i h w", w=HP)
                banks = [
                    psum.tile([128, ROWS, H], F32, tag="ps", name=f"ps{kt}_{n}_{ch}")
                    for ch in range(NCHUNK)
                ]
                # tap-major: evictions complete sooner after the closing
                # matmuls than chunk-major
                for t9 in range(9):
                    dy, dx = divmod(t9, 3)
                    for ch in range(NCHUNK):
                        r0 = ROWS * ch + dy
                        nc.tensor.matmul(
                            banks[ch][:],
                            wfp8[:, t9, :, kt * 128 : (kt + 1) * 128],
                            xpv[:, :, r0 : r0 + ROWS, dx : dx + H],
                            start=(t9 == 0),
                            stop=(t9 == 8),
                            perf_mode=mybir.MatmulPerfMode.DoubleRow,
                        )
                for ch in range(NCHUNK):
                    col = n * NCHUNK + ch
                    psv = banks[ch][:]
                    # evict valid columns to fp16 (exact) + per-chunk sum (DVE)
                    nc.vector.tensor_scalar(
                        out=ysb[kt][:, n, ch],
                        in0=psv,
                        scalar1=1.0,
                        scalar2=0.0,
                        op0=ALU.mult,
                        op1=ALU.add,
                        accum_out=sumc[:, kt, col : col + 1],
                    )
                    # sum of squares on ACT
                    sqs = evpool.tile([128, ROWS, H], F32, tag="sqs")
                    nc.scalar.activation(
                        out=sqs[:],
                        in_=psv,
                        func=AF.Square,
                        accum_out=sqc[:, kt, col : col + 1],
                    )
            # fold this kt's stats on DVE the moment its last eviction lands
            nc.vector.tensor_scalar(
                out=fold_scr[:],
                in0=sumc[:, kt, :],
                scalar1=1.0,
                scalar2=0.0,
                op0=ALU.mult,
                op1=ALU.add,
                accum_out=loc[kt][:, 0:1],
            )
            nc.vector.tensor_scalar(
                out=fold_scr[:],
                in0=sqc[:, kt, :],
                scalar1=1.0,
                scalar2=0.0,
                op0=ALU.mult,
                op1=ALU.add,
                accum_out=loc[kt][:, 1:2],
            )

        # pre-warm the ACT rsqrt table (in ACT queue order: after the squares)
        tblw = singles.tile([128, 1], F32)
        nc.scalar.activation(
            out=tblw[:], in_=eps_t[:], func=AF.Abs_reciprocal_sqrt, bias=eps_t[:]
        )

        # ---------------- per-kt stats exchange over RDMA ----------------
        # Single-phase all-to-all per kt: 7 single-slot sends (slot d ->
        # tpb^d, disjoint DMA-lane pairs, shared remote sem: +2 per arrival
        # -> one wait >= 14). Descriptors encode ADDRESSES only, so all 14
        # are pre-generated against staging tiles while pass 1 runs; each
        # kt's send critical copies loc->staging and fires one trigger(7).
        # gpsimd order pregen/send0/send1/recv0/recv1 keeps this core's kt1
        # send independent of kt0 peer arrivals. No entry barrier: remote
        # writes land long after launch while peers clear sems in the first
        # ~10us; the unwaited prelude AllGather registered below keeps NRT
        # launches synchronized. All inside tile_critical so Tile's
        # scheduling sim doesn't try (and fail) to satisfy the cross-core
        # sem waits.
        sloc = [singles.tile([128, 2], F32, name=f"a2a_src{kt}") for kt in range(2)]
        rall = [
            singles.tile([128, 7, 2], F32, name=f"a2a_rbuf{kt}") for kt in range(2)
        ]
        gstat = [singles.tile([128, 2], F32, name=f"a2a_g{kt}") for kt in range(2)]
        mv = [singles.tile([128, 2], F32, name=f"a2a_mv{kt}") for kt in range(2)]
        m2scr = singles.tile([128, 1], F32)
        rsem = [nc.alloc_semaphore(name=f"a2a_r{kt}") for kt in range(2)]
        lsem = nc.alloc_semaphore(name="a2a_l")
        psem = nc.alloc_semaphore(name="a2a_p")

        with tc.tile_critical(
            sync_engine=mybir.EngineType.Pool, no_gpsimd_drain=True
        ):
            for kt in range(2):
                for d in range(1, 8):
                    rdests = [None] * 8
                    rdests[d] = (0, d)
                    nc.gpsimd.remote_dma_broadcast(
                        out_ap=rall[kt][:, d - 1, :],
                        in_ap=sloc[kt][:],
                        remote_sem=rsem[kt],
                        local_sem=lsem,
                        rdests=rdests,
                    ).then_inc(psem, 1)
            nc.gpsimd.wait_ge(psem, 14)

        # send criticals: kt0 leaves kt1's 7 descriptors pending -> no drain
        for kt in range(2):
            with tc.tile_critical(
                sync_engine=mybir.EngineType.Pool, no_gpsimd_drain=(kt == 0)
            ):
                nc.gpsimd.tensor_scalar(
                    out=sloc[kt][:],
                    in0=loc[kt][:],
                    scalar1=1.0,
                    scalar2=0.0,
                    op0=ALU.mult,
                    op1=ALU.add,
                )
                nc.gpsimd.trigger_dma(count=7)

        def recv_and_finalize(kt):
            with tc.tile_critical(sync_engine=mybir.EngineType.Pool):
                nc.gpsimd.wait_ge(rsem[kt], 14)
                r = rall[kt]
                nc.gpsimd.tensor_add(
                    out=r[:, 0:3, :], in0=r[:, 0:3, :], in1=r[:, 3:6, :]
                )
                nc.gpsimd.tensor_add(out=r[:, 0, :], in0=r[:, 0, :], in1=r[:, 1, :])
                nc.gpsimd.tensor_add(out=r[:, 0, :], in0=r[:, 0, :], in1=r[:, 2, :])
                nc.gpsimd.tensor_add(out=r[:, 0, :], in0=r[:, 0, :], in1=r[:, 6, :])
                nc.gpsimd.tensor_add(
                    out=gstat[kt][:], in0=r[:, 0, :], in1=loc[kt][:]
                )
                # mean/var scaling on gpsimd too: saves a cross-engine hop
                nc.gpsimd.tensor_scalar(
                    out=mv[kt][:],
                    in0=gstat[kt][:],
                    scalar1=inv_cnt,
                    scalar2=None,
                    op0=ALU.mult,
                )
                nc.gpsimd.tensor_mul(
                    out=m2scr[:], in0=mv[kt][:, 0:1], in1=mv[kt][:, 0:1]
                )
                nc.gpsimd.tensor_sub(
                    out=mv[kt][:, 1:2], in0=mv[kt][:, 1:2], in1=m2scr[:]
                )

        scl = [singles.tile([128, 1], F32, name=f"scl{kt}") for kt in range(2)]
        bia = [singles.tile([128, 1], F32, name=f"bia{kt}") for kt in range(2)]
        rstd = [singles.tile([128, 1], F32, name=f"rstd{kt}") for kt in range(2)]

        def scale_bias(kt):
            nc.scalar.activation(
                out=rstd[kt][:],
                in_=mv[kt][:, 1:2],
                func=AF.Abs_reciprocal_sqrt,
                bias=eps_t[:],
            )
            nc.vector.tensor_mul(
                out=scl[kt][:], in0=gam[:, kt : kt + 1], in1=rstd[kt][:]
            )
            nc.vector.tensor_mul(
                out=bia[kt][:], in0=mv[kt][:, 0:1], in1=scl[kt][:]
            )
            nc.vector.tensor_sub(
                out=bia[kt][:], in0=bet[:, kt : kt + 1], in1=bia[kt][:]
            )

        # -------- pass 2: affine + clip + DMA out, streamed per kt --------
        NFULL = NCHUNK * ROWS * H  # 3136
        NHALF = NFULL // 2

        def pass2(kt, halves):
            """Affine+clip+store for one kt. halves=True splits each image in
            two for a faster first-DMA in the kt1 tail. Affines alternate
            DVE/ACT; clips on DVE; DMAs alternate the sync/scalar queues."""
            unit = 0
            for n in range(nimg):
                ysrc = ysb[kt][:, n].rearrange("p a b c -> p (a b c)")
                nparts = 2 if halves else 1
                for hf in range(nparts):
                    sl = slice(hf * NHALF, (hf + 1) * NHALF) if halves else slice(
                        0, NFULL
                    )
                    ob = (obpool2 if halves else obpool).tile(
                        [128, NHALF if halves else NFULL], F16, tag=f"ob{kt}"
                    )
                    if unit % 2 == 0:
                        nc.vector.tensor_scalar(
                            out=ob[:],
                            in0=ysrc[:, sl],
                            scalar1=scl[kt][:],
                            scalar2=bia[kt][:],
                            op0=ALU.mult,
                            op1=ALU.add,
                        )
                    else:
                        nc.scalar.activation(
                            out=ob[:],
                            in_=ysrc[:, sl],
                            func=AF.Identity,
                            bias=bia[kt][:],
                            scale=scl[kt][:],
                        )
                    nc.vector.tensor_scalar(
                        out=ob[:],
                        in0=ob[:],
                        scalar1=1.0,
                        scalar2=-1.0,
                        op0=ALU.min,
                        op1=ALU.max,
                    )
                    dma_eng = nc.sync if unit % 2 == 0 else nc.scalar
                    if halves:
                        obv = ob[:].rearrange("p (a b) -> p a b", b=H)
                        dma_eng.dma_start(
                            out=out_h[
                                n,
                                kt * 128 : (kt + 1) * 128,
                                hf * HH : (hf + 1) * HH,
                                :,
                            ],
                            in_=obv[:],
                        )
                    else:
                        obv = ob[:].rearrange("p (a b) -> p a b", b=H)
                        dma_eng.dma_start(
                            out=out_h[n, kt * 128 : (kt + 1) * 128, :, :],
                            in_=obv[:],
                        )
                    unit += 1

        recv_and_finalize(0)
        scale_bias(0)
        pass2(0, halves=False)
        recv_and_finalize(1)
        scale_bias(1)
        pass2(1, halves=True)

    # Register the kernel-entry barrier replica groups WITHOUT emitting a
    # wait: compile() then inserts a 1-byte prelude AllGather and sets
    # has_collectives, which makes NRT bring up global comm and launch the 8
    # cores synchronized (without any collective in the NEFF the cores launch
    # ms-staggered). Nobody waits on it, so ncfw's 65-150us cold start stays
    # off the critical path entirely.
    nc._bir_kernel_barrier_sem_replica_groups.extend([set(range(n_cores))])

    nc.compile()
    return nc


def prep_x(x):
    """Host prep: x [N,C,H,H] f32 -> padded binarized frames
    [N, c_lo=128, c_hi=2, SPPAD] fp8 (+-1, zero borders)."""
    import ml_dtypes

    n = x.shape[0]
    sign = np.where(np.asarray(x) >= 0, np.int8(1), np.int8(-1))
    arr = np.zeros((n, 128, 2, SPPAD), np.int8)
    view = arr[:, :, :, :SP].reshape(n, 128, 2, HP, HP)
    view[:, :, :, 1 : 1 + H, 1 : 1 + H] = sign.reshape(
        n, 2, 128, H, H
    ).transpose(0, 2, 1, 3, 4)
    return arr.astype(ml_dtypes.float8_e4m3)


def prep_w(W):
    """Host prep: W [K,C,3,3] f32 -> binarized (+-0.5)
    [c_lo=128, tap=9, c_hi=2, K] fp8."""
    import ml_dtypes

    wb = np.where(np.asarray(W) >= 0, np.float32(0.5), np.float32(-0.5))
    wt = wb.transpose(1, 2, 3, 0).reshape(C, 9, K)  # [c, t, k]
    # [c_hi, c_lo, t, k] -> [c_lo, t, c_hi, k]
    return np.ascontiguousarray(
        wt.reshape(2, 128, 9, K).transpose(1, 2, 0, 3)
    ).astype(ml_dtypes.float8_e4m3)


def _ensure_ntff_hooks():
    """Make run_bass_kernel_spmd's trace path importable on images whose
    antenv lacks axon_hooks (bass_utils hard-imports it when BASS_TRACE is
    set). Registers the real ctypes hook when available, else a None hook
    (bass_utils then logs and skips tracing instead of crashing)."""
    import sys
    import types

    try:
        import antenv
    except ImportError:
        return
    if hasattr(antenv, "axon_hooks") or "antenv.axon_hooks" in sys.modules:
        return
    hook = None
    try:
        from trn_agent_boot.trn_boot import _ntff_profile_via_ctypes

        hook = _ntff_profile_via_ctypes("/opt/axon/libaxon_pjrt.so")
    except Exception:
        hook = None
    mod = types.ModuleType("antenv.axon_hooks")
    mod.get_axon_ntff_profile_hook = lambda: hook
    mod.set_axon_ntff_profile_hook = lambda h: None
    sys.modules["antenv.axon_hooks"] = mod
    antenv.axon_hooks = mod


_ensure_ntff_hooks()


_CACHE = {}


def _get_compiled():
    if "nc" not in _CACHE:
        _CACHE["nc"] = build(8, 4, 32)
    return _CACHE["nc"]


def _in_maps(x, W, gamma, beta, n_cores, nimg):
    w2 = prep_w(W)
    g2 = np.ascontiguousarray(np.asarray(gamma, np.float32).reshape(K, 1))
    b2 = np.ascontiguousarray(np.asarray(beta, np.float32).reshape(K, 1))
    xp = prep_x(x)
    return [
        {
            "x": np.ascontiguousarray(xp[c * nimg : (c + 1) * nimg]),
            "w": w2,
            "gamma": g2,
            "beta": b2,
        }
        for c in range(n_cores)
    ]


def kernel(x, W, gamma, beta):
    """Full-input entry point: shard batch over 8 cores, run SPMD, gather."""
    from concourse.bass_utils import run_bass_kernel_spmd

    n_cores, nimg = 8, 4
    nc = _get_compiled()
    res = run_bass_kernel_spmd(
        nc, _in_maps(x, W, gamma, beta, n_cores, nimg), core_ids=list(range(n_cores))
    )
    out = np.concatenate(
        [res.results[c]["out"] for c in range(n_cores)], axis=0
    ).astype(np.float32)
    return out


def run_traced(x, W, gamma, beta):
    """Like kernel() but with NTFF tracing; returns (out, BassKernelResults)."""
    from concourse.bass_utils import run_bass_kernel_spmd

    n_cores, nimg = 8, 4
    nc = _get_compiled()
    res = run_bass_kernel_spmd(
        nc,
        _in_maps(x, W, gamma, beta, n_cores, nimg),
        core_ids=list(range(n_cores)),
        trace=True,
    )
    out = np.concatenate(
        [res.results[c]["out"] for c in range(n_cores)], axis=0
    ).astype(np.float32)
    return out, res
